# revision 7
# baseline (speedup 1.0000x reference)
"""Trainium2 Bass kernel for CausalSelfAttention (B=4, T=2048, C=2048, H=16).

Sharding: 8 cores = 4 batches x 2 head-groups (8 heads each). Each core runs
the full pipeline for its (batch, head-group): RMSNorm -> QKV (bf16 matmul)
-> per-head QK RMSNorm + RoPE -> causal attention (fp32 softmax, no
max-subtraction; exact since QK-normed scores are bounded) -> output
projection over its head slice + 0.5*residual. Host sums the two head-group
partials per batch (row-sharded c_proj all-reduce done on host).
"""

import math
import time

import numpy as np
import ml_dtypes

import concourse.bacc as bacc
import concourse.mybir as mybir
import concourse.tile as tile
from concourse.masks import make_causal_mask

F32 = mybir.dt.float32
BF16 = mybir.dt.bfloat16
NPBF = ml_dtypes.bfloat16
AF = mybir.ActivationFunctionType
ALU = mybir.AluOpType
AX = mybir.AxisListType

B = 4
T = 2048
C = 2048
HL = 8  # heads per core
DH = 128
DLOC = HL * DH  # 1024 output channels per core for each of q/k/v
TT = T // 128  # token tiles
CT = C // 128  # channel tiles
WCH = 512  # qkv weight chunk width (free dim of qkv matmuls)
NCH = 3 * DLOC // WCH
EPS = 1.1920929e-07
SCALE = 1.0 / math.sqrt(DH)
NEG = -30000.0  # additive causal mask value (pre-softmax-scale)
N_CORES = 8


def _build_nc():
    nc = bacc.Bacc("TRN2", target_bir_lowering=False)

    x_d = nc.dram_tensor("x", [T, C], F32, kind="ExternalInput")
    wqkv_d = nc.dram_tensor("wqkv", [128, CT * 3 * DLOC], BF16, kind="ExternalInput")
    wproj_d = nc.dram_tensor("wproj", [128, HL * C], BF16, kind="ExternalInput")
    cos_d = nc.dram_tensor("cos", [128, TT * 64], F32, kind="ExternalInput")
    sin_d = nc.dram_tensor("sin", [128, TT * 64], F32, kind="ExternalInput")
    y_d = nc.dram_tensor("y", [T, C], F32, kind="ExternalOutput")

    x_v = x_d.ap().rearrange("(tt p) c -> tt p c", p=128)
    wqkv_v = wqkv_d.ap().rearrange("p (ct d) -> p ct d", ct=CT)
    wproj_v = wproj_d.ap().rearrange("p (h c) -> p h c", h=HL)
    y_v = y_d.ap().rearrange("(tt p) c -> tt p c", p=128)

    with tile.TileContext(nc) as tc:
        consts = tc.alloc_tile_pool(name="consts", bufs=1)
        mask_sb = consts.tile([128, 128], F32)
        make_causal_mask(nc, mask_sb[:], mask_val=NEG)
        eps_sb = consts.tile([128, 1], F32)
        nc.gpsimd.memset(eps_sb[:], EPS)
        cos_sb = consts.tile([128, TT, 64], F32)
        nc.gpsimd.dma_start(cos_sb[:], cos_d.ap().rearrange("p (tt j) -> p tt j", tt=TT))
        sin_sb = consts.tile([128, TT, 64], F32)
        nc.gpsimd.dma_start(sin_sb[:], sin_d.ap().rearrange("p (tt j) -> p tt j", tt=TT))

        # ---------------- phase A1: x RMSNorm + transpose to [c, t] ----------
        xt_pool = tc.alloc_tile_pool(name="xt", bufs=1)
        xT = xt_pool.tile([128, CT, T], BF16)
        a1 = tc.alloc_tile_pool(name="a1", bufs=3)
        for tt in range(TT):
            xf = a1.tile([128, C], F32, tag="xf")
            nc.gpsimd.dma_start(xf[:], x_v[tt])
            sq = a1.tile([128, C], F32, tag="sq")
            ssum = a1.tile([128, 1], F32, tag="ssum")
            nc.vector.tensor_mul(sq[:], xf[:], xf[:])
            nc.vector.reduce_sum(ssum[:], sq[:], axis=AX.X)
            rstd = a1.tile([128, 1], F32, tag="rstd")
            nc.scalar.activation(rstd[:], ssum[:], AF.Sqrt, bias=eps_sb[:], scale=1.0 / C)
            nc.vector.reciprocal(rstd[:], rstd[:])
            xn = a1.tile([128, C], BF16, tag="xn")
            nc.vector.tensor_scalar_mul(xn[:], xf[:], rstd[:])
            for ct in range(CT):
                nc.sync.dma_start_transpose(
                    xT[:, ct, tt * 128 : (tt + 1) * 128],
                    xn[:, ct * 128 : (ct + 1) * 128],
                )
        a1.release()

        # ---------------- phase A2: QKV = xn @ wqkvT (token-major) -----------
        v_pool = tc.alloc_tile_pool(name="v", bufs=1, side="right")
        v_sb = v_pool.tile([128, TT, HL, DH], BF16)
        qk_pool = tc.alloc_tile_pool(name="qk", bufs=1, side="right")
        q_sb = qk_pool.tile([128, TT, HL, DH], BF16)
        k_sb = qk_pool.tile([128, TT, HL, DH], BF16)
        w_pool = tc.alloc_tile_pool(name="w", bufs=2)
        psA = tc.alloc_tile_pool(name="psA", bufs=4, space="PSUM")
        dests = (q_sb, k_sb, v_sb)
        per_mat = DLOC // WCH  # chunks per matrix
        hch = WCH // DH  # heads per chunk
        for ch in range(NCH):
            wch = w_pool.tile([128, CT, WCH], BF16, tag="wch")
            nc.gpsimd.dma_start(wch[:], wqkv_v[:, :, ch * WCH : (ch + 1) * WCH])
            for tt in range(TT):
                ps = psA.tile([128, WCH], F32, tag="psA")
                for ct in range(CT):
                    nc.tensor.matmul(
                        ps[:],
                        lhsT=xT[:, ct, tt * 128 : (tt + 1) * 128],
                        rhs=wch[:, ct, :],
                        start=(ct == 0),
                        stop=(ct == CT - 1),
                    )
                mat, ci = divmod(ch, per_mat)
                nc.scalar.copy(
                    dests[mat][:, tt, ci * hch : (ci + 1) * hch, :],
                    ps[:].rearrange("p (h d) -> p h d", h=hch),
                )
        w_pool.release()
        xt_pool.release()

        # ---------------- phase A3: QK head-RMSNorm + RoPE + transpose -------
        qt_pool = tc.alloc_tile_pool(name="qt", bufs=1)
        qT = qt_pool.tile([128, HL, T], BF16)
        kT = qt_pool.tile([128, HL, T], BF16)
        rp = tc.alloc_tile_pool(name="rope", bufs=2)
        for tt in range(TT):
            for src, dstT in ((q_sb, qT), (k_sb, kT)):
                blk = src[:, tt]  # [128, HL, DH] bf16
                sqr = rp.tile([128, HL, DH], F32, tag="sqr")
                nc.vector.tensor_mul(sqr[:], blk, blk)
                ssum = rp.tile([128, HL], F32, tag="ssum")
                nc.vector.reduce_sum(ssum[:], sqr[:], axis=AX.X)
                rr = rp.tile([128, HL], F32, tag="rr")
                nc.scalar.activation(rr[:], ssum[:], AF.Sqrt, bias=eps_sb[:], scale=1.0 / DH)
                nc.vector.reciprocal(rr[:], rr[:])
                rrb = rr[:, :, None].to_broadcast((128, HL, 64))
                cosb = cos_sb[:, tt, None, :].to_broadcast((128, HL, 64))
                sinb = sin_sb[:, tt, None, :].to_broadcast((128, HL, 64))
                x1 = blk[:, :, 0:64]
                x2 = blk[:, :, 64:128]
                t1 = rp.tile([128, HL, 64], F32, tag="t1")
                t2 = rp.tile([128, HL, 64], F32, tag="t2")
                rot = rp.tile([128, HL, DH], BF16, tag="rot")
                nc.vector.tensor_mul(t1[:], x1, cosb)
                nc.vector.tensor_mul(t2[:], x2, sinb)
                nc.vector.tensor_add(t1[:], t1[:], t2[:])
                nc.vector.tensor_mul(rot[:, :, 0:64], t1[:], rrb)
                nc.vector.tensor_mul(t1[:], x2, cosb)
                nc.vector.tensor_mul(t2[:], x1, sinb)
                nc.vector.tensor_sub(t1[:], t1[:], t2[:])
                nc.vector.tensor_mul(rot[:, :, 64:128], t1[:], rrb)
                for h in range(HL):
                    nc.sync.dma_start_transpose(
                        dstT[:, h, tt * 128 : (tt + 1) * 128], rot[:, h, :]
                    )
        rp.release()
        qk_pool.release()
        psA.release()

        # ---------------- phase B: causal attention per head -----------------
        ot_pool = tc.alloc_tile_pool(name="ot", bufs=1)
        oT = ot_pool.tile([128, HL, T], BF16)
        wp_pool = tc.alloc_tile_pool(name="wp", bufs=1)
        wp_sb = wp_pool.tile([128, HL, C], BF16)
        nc.gpsimd.dma_start(wp_sb[:], wproj_v)
        bp = tc.alloc_tile_pool(name="bp", bufs=2)
        psS = tc.alloc_tile_pool(name="psS", bufs=3, space="PSUM")
        psO = tc.alloc_tile_pool(name="psO", bufs=2, space="PSUM")
        for h in range(HL):
            for qb in range(TT):
                q0 = qb * 128
                kend = q0 + 128
                nch = (kend + 511) // 512
                p_sb = bp.tile([128, T], BF16, tag="p")
                sums = bp.tile([128, 4], F32, tag="sums")
                for ci in range(nch):
                    c0 = ci * 512
                    w = min(512, kend - c0)
                    ps = psS.tile([128, 512], F32, tag="s")
                    nc.tensor.matmul(
                        ps[:, :w],
                        lhsT=qT[:, h, q0 : q0 + 128],
                        rhs=kT[:, h, c0 : c0 + w],
                        start=True,
                        stop=True,
                    )
                    if c0 + w == kend:
                        nc.vector.tensor_add(
                            ps[:, w - 128 : w], ps[:, w - 128 : w], mask_sb[:]
                        )
                    nc.scalar.activation(
                        p_sb[:, c0 : c0 + w],
                        ps[:, :w],
                        AF.Exp,
                        scale=SCALE,
                        accum_out=sums[:, ci : ci + 1],
                    )
                rec = bp.tile([128, 1], F32, tag="rec")
                if nch > 1:
                    tot = bp.tile([128, 1], F32, tag="tot")
                    nc.vector.reduce_sum(tot[:], sums[:, :nch], axis=AX.X)
                    nc.vector.reciprocal(rec[:], tot[:])
                else:
                    nc.vector.reciprocal(rec[:], sums[:, 0:1])
                nc.vector.tensor_scalar_mul(p_sb[:, :kend], p_sb[:, :kend], rec[:])
                pT = bp.tile([128, TT, 128], BF16, tag="pT")
                for j in range(qb + 1):
                    nc.sync.dma_start_transpose(
                        pT[:, j, :], p_sb[:, j * 128 : (j + 1) * 128]
                    )
                po = psO.tile([128, 128], F32, tag="o")
                for j in range(qb + 1):
                    nc.tensor.matmul(
                        po[:],
                        lhsT=v_sb[:, j, h, :],
                        rhs=pT[:, j, :],
                        start=(j == 0),
                        stop=(j == qb),
                    )
                nc.vector.tensor_copy(oT[:, h, q0 : q0 + 128], po[:])
        bp.release()
        psO.release()
        psS.release()
        v_pool.release()

        # ---------------- phase C: out = 0.5*residual + oT.T @ wprojT --------
        cp = tc.alloc_tile_pool(name="cp", bufs=3)
        psP = tc.alloc_tile_pool(name="psP", bufs=3, space="PSUM")
        for tt in range(TT):
            res = cp.tile([128, C], F32, tag="res")
            nc.gpsimd.dma_start(res[:], x_v[tt])
            outsb = cp.tile([128, C], F32, tag="out")
            nc.vector.tensor_scalar_mul(outsb[:], res[:], 0.5)
            for cc in range(4):
                pp = psP.tile([128, 512], F32, tag="pp")
                for h in range(HL):
                    nc.tensor.matmul(
                        pp[:],
                        lhsT=oT[:, h, tt * 128 : (tt + 1) * 128],
                        rhs=wp_sb[:, h, cc * 512 : (cc + 1) * 512],
                        start=(h == 0),
                        stop=(h == HL - 1),
                    )
                nc.vector.tensor_add(
                    outsb[:, cc * 512 : (cc + 1) * 512],
                    outsb[:, cc * 512 : (cc + 1) * 512],
                    pp[:],
                )
            nc.gpsimd.dma_start(y_v[tt], outsb[:])
        cp.release()
        psP.release()
        wp_pool.release()
        ot_pool.release()
        qt_pool.release()
        consts.release()

    nc.compile()
    return nc


# ----------------------------------------------------------------------------
# host side: input prep, cached PJRT runner, timing
# ----------------------------------------------------------------------------

def _rope_tables():
    inv_freq = 1.0 / (10000.0 ** (np.arange(0, DH, 2, dtype=np.float32) / DH))
    t = np.arange(T, dtype=np.float32)
    freqs = np.outer(t, inv_freq).astype(np.float32)
    return np.cos(freqs).astype(np.float32), np.sin(freqs).astype(np.float32)


def _tile_rows(a):
    """[T, W] -> [128, T//128 * W] with partition = token-within-tile."""
    tt, w = a.shape[0] // 128, a.shape[1]
    return np.ascontiguousarray(
        a.reshape(tt, 128, w).transpose(1, 0, 2).reshape(128, tt * w)
    )


class _Runner:
    def __init__(self):
        import jax

        from concourse import bass2jax
        from concourse.bass2jax import _bass_exec_p, install_neuronx_cc_hook

        t0 = time.time()
        self.jax = jax
        nc = _build_nc()
        print(f"[kernel] bass build+compile passes: {time.time() - t0:.1f}s", flush=True)
        self.nc = nc
        install_neuronx_cc_hook()

        partition_name = (
            nc.partition_id_tensor.name if nc.partition_id_tensor else None
        )
        in_names: list[str] = []
        out_names: list[str] = []
        out_avals = []
        zero_shapes = []
        for alloc in nc.m.functions[0].allocations:
            if not isinstance(alloc, mybir.MemoryLocationSet):
                continue
            name = alloc.memorylocations[0].name
            if alloc.kind == "ExternalInput":
                if name != partition_name:
                    in_names.append(name)
            elif alloc.kind == "ExternalOutput":
                shape = tuple(alloc.tensor_shape)
                dtype = mybir.dt.np(alloc.dtype)
                out_names.append(name)
                out_avals.append(jax.core.ShapedArray(shape, dtype))
                zero_shapes.append((shape, dtype))
        n_params = len(in_names)
        n_outs = len(out_names)
        in_names = in_names + out_names
        if partition_name is not None:
            in_names.append(partition_name)
        self.in_names = in_names
        self.n_params = n_params
        self.out_names = out_names
        self.out_avals = out_avals
        self.zero_shapes = zero_shapes

        from jax.sharding import Mesh, PartitionSpec, NamedSharding
        from jax.experimental.shard_map import shard_map

        devices = jax.devices()[:N_CORES]
        assert len(devices) == N_CORES
        self.mesh = Mesh(np.asarray(devices), ("core",))
        self.sharding = NamedSharding(self.mesh, PartitionSpec("core"))

        def _body(*args):
            operands = list(args)
            if partition_name is not None:
                operands.append(bass2jax.partition_id_tensor())
            outs = _bass_exec_p.bind(
                *operands,
                out_avals=tuple(out_avals),
                in_names=tuple(in_names),
                out_names=tuple(out_names),
                lowering_input_output_aliases=(),
                sim_require_finite=True,
                sim_require_nnan=True,
                nc=nc,
            )
            return tuple(outs)

        donate = tuple(range(n_params, n_params + n_outs))
        in_specs = (PartitionSpec("core"),) * (n_params + n_outs)
        out_specs = (PartitionSpec("core"),) * n_outs
        self.sharded = jax.jit(
            shard_map(
                _body,
                mesh=self.mesh,
                in_specs=in_specs,
                out_specs=out_specs,
                check_rep=False,
            ),
            donate_argnums=donate,
            keep_unused=True,
        )

        import jax.numpy as jnp

        def _mk_zeros():
            return tuple(
                jnp.zeros((N_CORES * s[0], *s[1:]), d) for s, d in zero_shapes
            )

        self.zeros_fn = jax.jit(
            _mk_zeros, out_shardings=(self.sharding,) * n_outs
        )
        self.dev_inputs = None

    def set_inputs(self, in_maps):
        """in_maps: list of 8 dicts name->np array. Concats + puts on device."""
        concat = [
            np.concatenate(
                [np.asarray(m[name]) for m in in_maps], axis=0
            )
            for name in self.in_names[: self.n_params]
        ]
        self.dev_inputs = [
            self.jax.device_put(a, self.sharding) for a in concat
        ]

    def run(self):
        outs = self.sharded(*self.dev_inputs, *self.zeros_fn())
        return outs

    def run_np(self):
        outs = self.run()
        return [
            {
                name: np.asarray(outs[i]).reshape(
                    N_CORES, *self.out_avals[i].shape
                )[c]
                for i, name in enumerate(self.out_names)
            }
            for c in range(N_CORES)
        ]

    def benchmark(self, iters=10):
        # warmup (also triggers NEFF compile on first call)
        self.run()[0].block_until_ready()
        zero_sets = [self.zeros_fn() for _ in range(iters)]
        for z in zero_sets:
            z[0].block_until_ready()
        t0 = time.perf_counter()
        outs = None
        for i in range(iters):
            outs = self.sharded(*self.dev_inputs, *zero_sets[i])
        outs[0].block_until_ready()
        t1 = time.perf_counter()
        return (t1 - t0) / iters


_RUNNER = None


def _get_runner():
    global _RUNNER
    if _RUNNER is None:
        _RUNNER = _Runner()
    return _RUNNER


def _prep_in_maps(residual, wq, wk, wv, wproj):
    residual = np.asarray(residual, dtype=np.float32)
    cos, sin = _rope_tables()
    cos_arr = _tile_rows(cos)
    sin_arr = _tile_rows(sin)
    per_g = {}
    for g in range(2):
        sl = slice(g * DLOC, (g + 1) * DLOC)
        wqkvT = np.concatenate(
            [np.asarray(wq)[sl], np.asarray(wk)[sl], np.asarray(wv)[sl]], axis=0
        ).T  # [C, 3*DLOC]
        wqkv_arr = (
            wqkvT.reshape(CT, 128, 3 * DLOC)
            .transpose(1, 0, 2)
            .reshape(128, CT * 3 * DLOC)
            .astype(NPBF)
        )
        wprojT = np.asarray(wproj)[:, sl].T  # [DLOC, C]
        wproj_arr = (
            wprojT.reshape(HL, 128, C)
            .transpose(1, 0, 2)
            .reshape(128, HL * C)
            .astype(NPBF)
        )
        per_g[g] = (np.ascontiguousarray(wqkv_arr), np.ascontiguousarray(wproj_arr))
    in_maps = []
    for core in range(N_CORES):
        b, g = divmod(core, 2)
        wqkv_arr, wproj_arr = per_g[g]
        in_maps.append(
            {
                "x": np.ascontiguousarray(residual[b]),
                "wqkv": wqkv_arr,
                "wproj": wproj_arr,
                "cos": cos_arr,
                "sin": sin_arr,
            }
        )
    return in_maps


def kernel(residual, wq, wk, wv, wproj):
    runner = _get_runner()
    runner.set_inputs(_prep_in_maps(residual, wq, wk, wv, wproj))
    results = runner.run_np()
    out = np.empty((B, T, C), dtype=np.float32)
    for b in range(B):
        out[b] = results[2 * b]["y"] + results[2 * b + 1]["y"]
    return out


def benchmark(iters=10):
    assert _RUNNER is not None and _RUNNER.dev_inputs is not None
    return _RUNNER.benchmark(iters)


# revision 8
# speedup vs baseline: 3.7970x; 3.7970x over previous
"""Trainium2 Bass kernel for CausalSelfAttention (B=4, T=2048, C=2048, H=16).

Sharding: 8 cores = 4 batches x 2 head-groups (8 heads each). Each core runs
the full pipeline for its (batch, head-group): RMSNorm -> QKV (bf16 matmul)
-> per-head QK RMSNorm + RoPE -> causal attention (fp32 softmax, no
max-subtraction; exact since QK-normed scores are bounded) -> output
projection over its head slice + 0.5*residual. Host sums the two head-group
partials per batch (row-sharded c_proj all-reduce done on host).
"""

import math
import time

import numpy as np
import ml_dtypes

import concourse.bacc as bacc
import concourse.mybir as mybir
import concourse.tile as tile
from concourse.masks import make_causal_mask

F32 = mybir.dt.float32
BF16 = mybir.dt.bfloat16
NPBF = ml_dtypes.bfloat16
AF = mybir.ActivationFunctionType
ALU = mybir.AluOpType
AX = mybir.AxisListType

B = 4
T = 2048
C = 2048
HL = 8  # heads per core
DH = 128
DLOC = HL * DH  # 1024 output channels per core for each of q/k/v
TT = T // 128  # token tiles
CT = C // 128  # channel tiles
WCH = 256  # qkv weight chunk width (free dim of qkv matmuls)
NCH = 3 * DLOC // WCH
EPS = 1.1920929e-07
SCALE = 1.0 / math.sqrt(DH)
NEG = -30000.0  # additive causal mask value (pre-softmax-scale)
N_CORES = 8


def _build_nc():
    nc = bacc.Bacc("TRN2", target_bir_lowering=False)

    x_d = nc.dram_tensor("x", [T, C], F32, kind="ExternalInput")
    wqkv_d = nc.dram_tensor("wqkv", [128, CT * 3 * DLOC], BF16, kind="ExternalInput")
    wproj_d = nc.dram_tensor("wproj", [128, HL * C], BF16, kind="ExternalInput")
    cos_d = nc.dram_tensor("cos", [128, TT * 64], F32, kind="ExternalInput")
    sin_d = nc.dram_tensor("sin", [128, TT * 64], F32, kind="ExternalInput")
    y_d = nc.dram_tensor("y", [T, C], F32, kind="ExternalOutput")

    x_v = x_d.ap().rearrange("(tt p) c -> tt p c", p=128)
    wqkv_v = wqkv_d.ap().rearrange("p (ct d) -> p ct d", ct=CT)
    wproj_v = wproj_d.ap().rearrange("p (h c) -> p h c", h=HL)
    y_v = y_d.ap().rearrange("(tt p) c -> tt p c", p=128)

    with tile.TileContext(nc) as tc:
        consts = tc.alloc_tile_pool(name="consts", bufs=1)
        mask_sb = consts.tile([128, 128], F32)
        make_causal_mask(nc, mask_sb[:], mask_val=NEG)
        eps_sb = consts.tile([128, 1], F32)
        nc.gpsimd.memset(eps_sb[:], EPS)
        cos_sb = consts.tile([128, TT, 64], F32)
        nc.gpsimd.dma_start(cos_sb[:], cos_d.ap().rearrange("p (tt j) -> p tt j", tt=TT))
        sin_sb = consts.tile([128, TT, 64], F32)
        nc.gpsimd.dma_start(sin_sb[:], sin_d.ap().rearrange("p (tt j) -> p tt j", tt=TT))

        # ---------------- phase A1: x RMSNorm + transpose to [c, t] ----------
        xt_pool = tc.alloc_tile_pool(name="xt", bufs=1)
        xT = xt_pool.tile([128, CT, T], BF16)
        a1 = tc.alloc_tile_pool(name="a1", bufs=3)
        for tt in range(TT):
            xf = a1.tile([128, C], F32, tag="xf")
            nc.gpsimd.dma_start(xf[:], x_v[tt])
            sq = a1.tile([128, C], F32, tag="sq")
            ssum = a1.tile([128, 1], F32, tag="ssum")
            nc.vector.tensor_mul(sq[:], xf[:], xf[:])
            nc.vector.reduce_sum(ssum[:], sq[:], axis=AX.X)
            rstd = a1.tile([128, 1], F32, tag="rstd")
            nc.scalar.activation(rstd[:], ssum[:], AF.Sqrt, bias=eps_sb[:], scale=1.0 / C)
            nc.vector.reciprocal(rstd[:], rstd[:])
            xn = a1.tile([128, C], BF16, tag="xn")
            nc.vector.tensor_scalar_mul(xn[:], xf[:], rstd[:])
            for ct in range(CT):
                nc.sync.dma_start_transpose(
                    xT[:, ct, tt * 128 : (tt + 1) * 128],
                    xn[:, ct * 128 : (ct + 1) * 128],
                )
        a1.release()

        # ---------------- phase A2: QKV = xn @ wqkvT (token-major) -----------
        v_pool = tc.alloc_tile_pool(name="v", bufs=1, side="right")
        v_sb = v_pool.tile([128, TT, HL, DH], BF16)
        qk_pool = tc.alloc_tile_pool(name="qk", bufs=1, side="right")
        q_sb = qk_pool.tile([128, TT, HL, DH], BF16)
        k_sb = qk_pool.tile([128, TT, HL, DH], BF16)
        w_pool = tc.alloc_tile_pool(name="w", bufs=2)
        psA = tc.alloc_tile_pool(name="psA", bufs=4, space="PSUM")
        dests = (q_sb, k_sb, v_sb)
        per_mat = DLOC // WCH  # chunks per matrix
        hch = WCH // DH  # heads per chunk
        for ch in range(NCH):
            wch = w_pool.tile([128, CT, WCH], BF16, tag="wch")
            nc.gpsimd.dma_start(wch[:], wqkv_v[:, :, ch * WCH : (ch + 1) * WCH])
            for tt in range(TT):
                ps = psA.tile([128, WCH], F32, tag="psA")
                for ct in range(CT):
                    nc.tensor.matmul(
                        ps[:],
                        lhsT=xT[:, ct, tt * 128 : (tt + 1) * 128],
                        rhs=wch[:, ct, :],
                        start=(ct == 0),
                        stop=(ct == CT - 1),
                    )
                mat, ci = divmod(ch, per_mat)
                nc.scalar.copy(
                    dests[mat][:, tt, ci * hch : (ci + 1) * hch, :],
                    ps[:].rearrange("p (h d) -> p h d", h=hch),
                )
        w_pool.release()
        xt_pool.release()

        # ---------------- phase A3: QK head-RMSNorm + RoPE + transpose -------
        qt_pool = tc.alloc_tile_pool(name="qt", bufs=1)
        qT = qt_pool.tile([128, HL, T], BF16)
        kT = qt_pool.tile([128, HL, T], BF16)
        rp = tc.alloc_tile_pool(name="rope", bufs=2)
        for tt in range(TT):
            for src, dstT in ((q_sb, qT), (k_sb, kT)):
                blk = src[:, tt]  # [128, HL, DH] bf16
                sqr = rp.tile([128, HL, DH], F32, tag="sqr")
                nc.vector.tensor_mul(sqr[:], blk, blk)
                ssum = rp.tile([128, HL], F32, tag="ssum")
                nc.vector.reduce_sum(ssum[:], sqr[:], axis=AX.X)
                rr = rp.tile([128, HL], F32, tag="rr")
                nc.scalar.activation(rr[:], ssum[:], AF.Sqrt, bias=eps_sb[:], scale=1.0 / DH)
                nc.vector.reciprocal(rr[:], rr[:])
                rrb = rr[:, :, None].to_broadcast((128, HL, 64))
                cosb = cos_sb[:, tt, None, :].to_broadcast((128, HL, 64))
                sinb = sin_sb[:, tt, None, :].to_broadcast((128, HL, 64))
                x1 = blk[:, :, 0:64]
                x2 = blk[:, :, 64:128]
                t1 = rp.tile([128, HL, 64], F32, tag="t1")
                t2 = rp.tile([128, HL, 64], F32, tag="t2")
                rot = rp.tile([128, HL, DH], BF16, tag="rot")
                nc.vector.tensor_mul(t1[:], x1, cosb)
                nc.vector.tensor_mul(t2[:], x2, sinb)
                nc.vector.tensor_add(t1[:], t1[:], t2[:])
                nc.vector.tensor_mul(rot[:, :, 0:64], t1[:], rrb)
                nc.vector.tensor_mul(t1[:], x2, cosb)
                nc.vector.tensor_mul(t2[:], x1, sinb)
                nc.vector.tensor_sub(t1[:], t1[:], t2[:])
                nc.vector.tensor_mul(rot[:, :, 64:128], t1[:], rrb)
                for h in range(HL):
                    nc.sync.dma_start_transpose(
                        dstT[:, h, tt * 128 : (tt + 1) * 128], rot[:, h, :]
                    )
        rp.release()
        qk_pool.release()
        psA.release()

        # ---------------- phase B: causal attention per head -----------------
        ot_pool = tc.alloc_tile_pool(name="ot", bufs=1)
        oT = ot_pool.tile([128, HL, T], BF16)
        wp_pool = tc.alloc_tile_pool(name="wp", bufs=1)
        wp_sb = wp_pool.tile([128, HL, C], BF16)
        nc.gpsimd.dma_start(wp_sb[:], wproj_v)
        bp = tc.alloc_tile_pool(name="bp", bufs=2)
        psS = tc.alloc_tile_pool(name="psS", bufs=3, space="PSUM")
        psO = tc.alloc_tile_pool(name="psO", bufs=2, space="PSUM")
        for h in range(HL):
            for qb in range(TT):
                q0 = qb * 128
                kend = q0 + 128
                nch = (kend + 511) // 512
                p_sb = bp.tile([128, T], BF16, tag="p")
                sums = bp.tile([128, 4], F32, tag="sums")
                for ci in range(nch):
                    c0 = ci * 512
                    w = min(512, kend - c0)
                    ps = psS.tile([128, 512], F32, tag="s")
                    nc.tensor.matmul(
                        ps[:, :w],
                        lhsT=qT[:, h, q0 : q0 + 128],
                        rhs=kT[:, h, c0 : c0 + w],
                        start=True,
                        stop=True,
                    )
                    if c0 + w == kend:
                        nc.vector.tensor_add(
                            ps[:, w - 128 : w], ps[:, w - 128 : w], mask_sb[:]
                        )
                    nc.scalar.activation(
                        p_sb[:, c0 : c0 + w],
                        ps[:, :w],
                        AF.Exp,
                        scale=SCALE,
                        accum_out=sums[:, ci : ci + 1],
                    )
                rec = bp.tile([128, 1], F32, tag="rec")
                if nch > 1:
                    tot = bp.tile([128, 1], F32, tag="tot")
                    nc.vector.reduce_sum(tot[:], sums[:, :nch], axis=AX.X)
                    nc.vector.reciprocal(rec[:], tot[:])
                else:
                    nc.vector.reciprocal(rec[:], sums[:, 0:1])
                nc.vector.tensor_scalar_mul(p_sb[:, :kend], p_sb[:, :kend], rec[:])
                pT = bp.tile([128, TT, 128], BF16, tag="pT")
                for j in range(qb + 1):
                    nc.sync.dma_start_transpose(
                        pT[:, j, :], p_sb[:, j * 128 : (j + 1) * 128]
                    )
                po = psO.tile([128, 128], F32, tag="o")
                for j in range(qb + 1):
                    nc.tensor.matmul(
                        po[:],
                        lhsT=v_sb[:, j, h, :],
                        rhs=pT[:, j, :],
                        start=(j == 0),
                        stop=(j == qb),
                    )
                nc.vector.tensor_copy(oT[:, h, q0 : q0 + 128], po[:])
        bp.release()
        psO.release()
        psS.release()
        v_pool.release()

        # ---------------- phase C: out = 0.5*residual + oT.T @ wprojT --------
        cp = tc.alloc_tile_pool(name="cp", bufs=3)
        psP = tc.alloc_tile_pool(name="psP", bufs=3, space="PSUM")
        for tt in range(TT):
            res = cp.tile([128, C], F32, tag="res")
            nc.gpsimd.dma_start(res[:], x_v[tt])
            outsb = cp.tile([128, C], F32, tag="out")
            nc.vector.tensor_scalar_mul(outsb[:], res[:], 0.5)
            for cc in range(4):
                pp = psP.tile([128, 512], F32, tag="pp")
                for h in range(HL):
                    nc.tensor.matmul(
                        pp[:],
                        lhsT=oT[:, h, tt * 128 : (tt + 1) * 128],
                        rhs=wp_sb[:, h, cc * 512 : (cc + 1) * 512],
                        start=(h == 0),
                        stop=(h == HL - 1),
                    )
                nc.vector.tensor_add(
                    outsb[:, cc * 512 : (cc + 1) * 512],
                    outsb[:, cc * 512 : (cc + 1) * 512],
                    pp[:],
                )
            nc.gpsimd.dma_start(y_v[tt], outsb[:])
        cp.release()
        psP.release()
        wp_pool.release()
        ot_pool.release()
        qt_pool.release()
        consts.release()

    nc.compile()
    return nc


# ----------------------------------------------------------------------------
# host side: input prep, cached PJRT runner, timing
# ----------------------------------------------------------------------------

def _rope_tables():
    inv_freq = 1.0 / (10000.0 ** (np.arange(0, DH, 2, dtype=np.float32) / DH))
    t = np.arange(T, dtype=np.float32)
    freqs = np.outer(t, inv_freq).astype(np.float32)
    return np.cos(freqs).astype(np.float32), np.sin(freqs).astype(np.float32)


def _tile_rows(a):
    """[T, W] -> [128, T//128 * W] with partition = token-within-tile."""
    tt, w = a.shape[0] // 128, a.shape[1]
    return np.ascontiguousarray(
        a.reshape(tt, 128, w).transpose(1, 0, 2).reshape(128, tt * w)
    )


class _Runner:
    def __init__(self):
        import jax

        from concourse import bass2jax
        from concourse.bass2jax import _bass_exec_p, install_neuronx_cc_hook

        t0 = time.time()
        self.jax = jax
        nc = _build_nc()
        print(f"[kernel] bass build+compile passes: {time.time() - t0:.1f}s", flush=True)
        self.nc = nc
        install_neuronx_cc_hook()

        partition_name = (
            nc.partition_id_tensor.name if nc.partition_id_tensor else None
        )
        in_names: list[str] = []
        out_names: list[str] = []
        out_avals = []
        zero_shapes = []
        for alloc in nc.m.functions[0].allocations:
            if not isinstance(alloc, mybir.MemoryLocationSet):
                continue
            name = alloc.memorylocations[0].name
            if alloc.kind == "ExternalInput":
                if name != partition_name:
                    in_names.append(name)
            elif alloc.kind == "ExternalOutput":
                shape = tuple(alloc.tensor_shape)
                dtype = mybir.dt.np(alloc.dtype)
                out_names.append(name)
                out_avals.append(jax.core.ShapedArray(shape, dtype))
                zero_shapes.append((shape, dtype))
        n_params = len(in_names)
        n_outs = len(out_names)
        in_names = in_names + out_names
        if partition_name is not None:
            in_names.append(partition_name)
        self.in_names = in_names
        self.n_params = n_params
        self.out_names = out_names
        self.out_avals = out_avals
        self.zero_shapes = zero_shapes

        from jax.sharding import Mesh, PartitionSpec, NamedSharding
        from jax.experimental.shard_map import shard_map

        devices = jax.devices()[:N_CORES]
        assert len(devices) == N_CORES
        self.mesh = Mesh(np.asarray(devices), ("core",))
        self.sharding = NamedSharding(self.mesh, PartitionSpec("core"))

        def _body(*args):
            operands = list(args)
            if partition_name is not None:
                operands.append(bass2jax.partition_id_tensor())
            outs = _bass_exec_p.bind(
                *operands,
                out_avals=tuple(out_avals),
                in_names=tuple(in_names),
                out_names=tuple(out_names),
                lowering_input_output_aliases=(),
                sim_require_finite=True,
                sim_require_nnan=True,
                nc=nc,
            )
            return tuple(outs)

        donate = tuple(range(n_params, n_params + n_outs))
        in_specs = (PartitionSpec("core"),) * (n_params + n_outs)
        out_specs = (PartitionSpec("core"),) * n_outs
        self.sharded = jax.jit(
            shard_map(
                _body,
                mesh=self.mesh,
                in_specs=in_specs,
                out_specs=out_specs,
                check_rep=False,
            ),
            donate_argnums=donate,
            keep_unused=True,
        )

        import jax.numpy as jnp

        def _mk_zeros():
            return tuple(
                jnp.zeros((N_CORES * s[0], *s[1:]), d) for s, d in zero_shapes
            )

        self.zeros_fn = jax.jit(
            _mk_zeros, out_shardings=(self.sharding,) * n_outs
        )
        self.dev_inputs = None

    def set_inputs(self, in_maps):
        """in_maps: list of 8 dicts name->np array. Concats + puts on device."""
        concat = [
            np.concatenate(
                [np.asarray(m[name]) for m in in_maps], axis=0
            )
            for name in self.in_names[: self.n_params]
        ]
        self.dev_inputs = [
            self.jax.device_put(a, self.sharding) for a in concat
        ]

    def run(self):
        outs = self.sharded(*self.dev_inputs, *self.zeros_fn())
        return outs

    def run_np(self):
        outs = self.run()
        return [
            {
                name: np.asarray(outs[i]).reshape(
                    N_CORES, *self.out_avals[i].shape
                )[c]
                for i, name in enumerate(self.out_names)
            }
            for c in range(N_CORES)
        ]

    def benchmark(self, iters=10):
        # warmup (also triggers NEFF compile on first call)
        self.run()[0].block_until_ready()
        zero_sets = [self.zeros_fn() for _ in range(iters)]
        for z in zero_sets:
            z[0].block_until_ready()
        t0 = time.perf_counter()
        outs = None
        for i in range(iters):
            outs = self.sharded(*self.dev_inputs, *zero_sets[i])
        outs[0].block_until_ready()
        t1 = time.perf_counter()
        return (t1 - t0) / iters


_RUNNER = None


def _get_runner():
    global _RUNNER
    if _RUNNER is None:
        _RUNNER = _Runner()
    return _RUNNER


def _prep_in_maps(residual, wq, wk, wv, wproj):
    residual = np.asarray(residual, dtype=np.float32)
    cos, sin = _rope_tables()
    cos_arr = _tile_rows(cos)
    sin_arr = _tile_rows(sin)
    per_g = {}
    for g in range(2):
        sl = slice(g * DLOC, (g + 1) * DLOC)
        wqkvT = np.concatenate(
            [np.asarray(wq)[sl], np.asarray(wk)[sl], np.asarray(wv)[sl]], axis=0
        ).T  # [C, 3*DLOC]
        wqkv_arr = (
            wqkvT.reshape(CT, 128, 3 * DLOC)
            .transpose(1, 0, 2)
            .reshape(128, CT * 3 * DLOC)
            .astype(NPBF)
        )
        wprojT = np.asarray(wproj)[:, sl].T  # [DLOC, C]
        wproj_arr = (
            wprojT.reshape(HL, 128, C)
            .transpose(1, 0, 2)
            .reshape(128, HL * C)
            .astype(NPBF)
        )
        per_g[g] = (np.ascontiguousarray(wqkv_arr), np.ascontiguousarray(wproj_arr))
    in_maps = []
    for core in range(N_CORES):
        b, g = divmod(core, 2)
        wqkv_arr, wproj_arr = per_g[g]
        in_maps.append(
            {
                "x": np.ascontiguousarray(residual[b]),
                "wqkv": wqkv_arr,
                "wproj": wproj_arr,
                "cos": cos_arr,
                "sin": sin_arr,
            }
        )
    return in_maps


def kernel(residual, wq, wk, wv, wproj):
    runner = _get_runner()
    runner.set_inputs(_prep_in_maps(residual, wq, wk, wv, wproj))
    results = runner.run_np()
    out = np.empty((B, T, C), dtype=np.float32)
    for b in range(B):
        out[b] = results[2 * b]["y"] + results[2 * b + 1]["y"]
    return out


def benchmark(iters=10):
    assert _RUNNER is not None and _RUNNER.dev_inputs is not None
    return _RUNNER.benchmark(iters)
